# revision 1
# baseline (speedup 1.0000x reference)
"""Trainium2 Bass kernel for ExpertsChooseMaskedExpand MoE routing.

Math (reference):
    xd[b,e,c,i] = sum_t x[b,t,(e,i)] * dmask[b,t,e,c]            (dispatch)
    y[b,e,c,o]  = sum_i xd[b,e,c,i] * w[e,o,i] + bias[o]         (expert mm)
    out[b,t,o]  = sum_{e,c} y[b,e,c,o] * cmb[b,t,e,c]            (combine)

Restructured (combine applied before the weight matmul — 155 GF total
instead of 215 GF, and the expert matmuls fuse into one K=2048 matmul):
    xd[b,e][c,j] = sum_t dmask[b,e][t,c] * xr[b,e][t,j]
    z[b,e][t,j]  = sum_c cmb[b,e][t,c] * xd[b,e][c,j]
    out[b][t,o]  = sum_{(e,j)} z[b][t,(e,j)] * wstack[(e,j),o] + s[b][t]*bias[o]
    where s[b][t] = sum_{e,c} cmb[b,t,e,c],  wstack[(e,j),o] = w[e,o,j]

Sharding: 8 cores = (batch b in 0..3) x (output half oh in 0..1). Each
core computes its exact out[b][:, oh*4096:(oh+1)*4096] slice - no
cross-core reduction. All matmuls run as float32r (fp22, full PE rate).
"""

import numpy as np

B, T, E, C = 4, 1024, 4, 512
IN, OUT = 2048, 8192
P = 128
TT = T // P          # 8  t-tiles
CT = C // P          # 4  c-tiles per expert
JT = 4               # j-tiles per expert (i = 512)
KT = E * JT          # 16 k-tiles for the fused matmul (K = 2048)
OH = OUT // 2        # 4096 output columns per core
OC = OH // 512       # 8  output chunks of 512

_CACHE = {}


def _build_nc():
    import concourse.mybir as mybir
    import concourse.tile as tile
    from concourse import bacc

    f32 = mybir.dt.float32
    f32r = mybir.dt.float32r

    nc = bacc.Bacc("TRN2", target_bir_lowering=False, debug=False, num_devices=8)
    x_t = nc.dram_tensor("x", (T, IN), f32r, kind="ExternalInput")
    dm_t = nc.dram_tensor("dm", (T, E, C), f32r, kind="ExternalInput")
    cT_t = nc.dram_tensor("cmbT", (E, C, T), f32r, kind="ExternalInput")
    wT_t = nc.dram_tensor("wT", (KT, P, OH), f32r, kind="ExternalInput")
    s_t = nc.dram_tensor("s", (1, T), f32r, kind="ExternalInput")
    b_t = nc.dram_tensor("bias", (1, OH), f32r, kind="ExternalInput")
    o_t = nc.dram_tensor("out", (T, OH), f32, kind="ExternalOutput")

    x_r = x_t.ap().rearrange("(tt p) f -> p tt f", p=P)        # [128, 8, 2048]
    dm_r = dm_t.ap().rearrange("(tt p) e c -> p tt e c", p=P)  # [128, 8, 4, 512]
    cT_r = cT_t.ap().rearrange("e (ct p) t -> p e ct t", p=P)  # [128, 4, 4, 1024]
    wT_r = wT_t.ap().rearrange("kt p o -> p kt o")             # [128, 16, 4096]
    o_r = o_t.ap().rearrange("(tt p) o -> p tt o", p=P)        # [128, 8, 4096]

    with tile.TileContext(nc) as tc:
        with (
            tc.tile_pool(name="persist", bufs=1) as persist,
            tc.tile_pool(name="psum", bufs=2, space="PSUM") as psum,
        ):
            zT = persist.tile([P, KT, T], f32r)       # 64 KiB/partition
            s_sb = persist.tile([1, T], f32r)
            bias_sb = persist.tile([1, OH], f32r)
            nc.sync.dma_start(s_sb, s_t.ap())
            nc.sync.dma_start(bias_sb, b_t.ap())

            # Phases 1+2: per-expert dispatch and combine
            with tc.tile_pool(name="exp", bufs=2) as exp:
                for e in range(E):
                    x_e = exp.tile([P, TT, 512], f32r, tag="x_e")
                    dm_e = exp.tile([P, TT, C], f32r, tag="dm_e")
                    c_e = exp.tile([P, CT, T], f32r, tag="c_e")
                    nc.sync.dma_start(x_e, x_r[:, :, e * 512 : (e + 1) * 512])
                    nc.sync.dma_start(dm_e, dm_r[:, :, e, :])
                    nc.sync.dma_start(c_e, cT_r[:, e, :, :])

                    # xd[c, j] = sum_t dm[t, c] * x[t, j]
                    xd_e = exp.tile([P, CT, 512], f32r, tag="xd_e")
                    for ct in range(CT):
                        ps1 = psum.tile([P, 512], f32, tag="ps1")
                        for tt in range(TT):
                            nc.tensor.matmul(
                                ps1,
                                dm_e[:, tt, ct * P : (ct + 1) * P],
                                x_e[:, tt, :],
                                start=(tt == 0),
                                stop=(tt == TT - 1),
                            )
                        nc.vector.tensor_copy(xd_e[:, ct, :], ps1)

                    # zT[j, t] = sum_c xd[c, j] * cmbT[c, t]
                    for th in range(2):
                        for jt in range(JT):
                            ps2 = psum.tile([P, 512], f32, tag="ps2")
                            for ct in range(CT):
                                nc.tensor.matmul(
                                    ps2,
                                    xd_e[:, ct, jt * P : (jt + 1) * P],
                                    c_e[:, ct, th * 512 : (th + 1) * 512],
                                    start=(ct == 0),
                                    stop=(ct == CT - 1),
                                )
                            nc.vector.tensor_copy(
                                zT[:, e * JT + jt, th * 512 : (th + 1) * 512], ps2
                            )

            # Phase 3: out[t, o] = sum_kt zT[kt][:, t].T @ w[kt] + s[t] * bias[o]
            with (
                tc.tile_pool(name="wp", bufs=2) as wp,
                tc.tile_pool(name="op", bufs=3) as op,
            ):
                for oc in range(OC):
                    w_oc = wp.tile([P, KT, 512], f32r, tag="w_oc")
                    nc.sync.dma_start(w_oc, wT_r[:, :, oc * 512 : (oc + 1) * 512])
                    for tt in range(TT):
                        ps3 = psum.tile([P, 512], f32, tag="ps3")
                        for kt in range(KT):
                            nc.tensor.matmul(
                                ps3,
                                zT[:, kt, tt * P : (tt + 1) * P],
                                w_oc[:, kt, :],
                                start=(kt == 0),
                                stop=False,
                            )
                        # rank-1 bias update: += s[t] * bias[o]
                        nc.tensor.matmul(
                            ps3,
                            s_sb[:, tt * P : (tt + 1) * P],
                            bias_sb[:, oc * 512 : (oc + 1) * 512],
                            start=False,
                            stop=True,
                        )
                        o_sb = op.tile([P, 512], f32, tag="o_sb")
                        nc.vector.tensor_copy(o_sb, ps3)
                        nc.sync.dma_start(o_r[:, tt, oc * 512 : (oc + 1) * 512], o_sb)

    nc.compile()
    return nc


def _get_nc():
    if "nc" not in _CACHE:
        _CACHE["nc"] = _build_nc()
    return _CACHE["nc"]


def _prep_in_maps(x, combine_array, dispatch_mask, weight, bias):
    x = np.ascontiguousarray(x, dtype=np.float32)
    cmb = np.ascontiguousarray(combine_array, dtype=np.float32)
    dm = np.ascontiguousarray(dispatch_mask, dtype=np.float32)
    weight = np.ascontiguousarray(weight, dtype=np.float32)
    bias = np.ascontiguousarray(bias, dtype=np.float32)

    # combine transposed to (B, E, C, T) so that C lands on partitions
    cmbT = np.ascontiguousarray(cmb.transpose(0, 2, 3, 1))
    s = cmb.sum(axis=(2, 3))  # (B, T)
    # wstack[(e,j), o] = w[e, o, j];  w = weight.reshape(E, OUT, IN//E)
    w = weight.reshape(E, OUT, IN // E)
    wstack = np.ascontiguousarray(w.transpose(0, 2, 1)).reshape(IN, OUT)
    wT = [
        np.ascontiguousarray(wstack[:, oh * OH : (oh + 1) * OH]).reshape(KT, P, OH)
        for oh in range(2)
    ]
    bias_h = [np.ascontiguousarray(bias[oh * OH : (oh + 1) * OH]) for oh in range(2)]

    in_maps = []
    for k in range(8):
        b, oh = k // 2, k % 2
        in_maps.append(
            {
                "x": x[b],
                "dm": dm[b],
                "cmbT": cmbT[b],
                "wT": wT[oh],
                "s": s[b : b + 1],
                "bias": bias_h[oh].reshape(1, OH),
            }
        )
    return in_maps


def run_spmd(in_maps, trace=False, **kwargs):
    from concourse.bass_utils import run_bass_kernel_spmd

    nc = _get_nc()
    return run_bass_kernel_spmd(
        nc, in_maps, core_ids=list(range(8)), trace=trace, **kwargs
    )


def kernel(x, combine_array, dispatch_mask, weight, bias, num_experts):
    assert int(num_experts) == E
    in_maps = _prep_in_maps(x, combine_array, dispatch_mask, weight, bias)
    res = run_spmd(in_maps)
    out = np.empty((B, T, OUT), dtype=np.float32)
    for k in range(8):
        b, oh = k // 2, k % 2
        out[b, :, oh * OH : (oh + 1) * OH] = res.results[k]["out"]
    return out
